# revision 13
# baseline (speedup 1.0000x reference)
"""Trainium2 Bass kernel for a linear-attention transformer block.

B=8, S=4096, E=512, NH=8, DH=64, HID=2048.
Sharding: data-parallel over batch — one batch element per NeuronCore, all
weights replicated, zero collectives.

Layouts are chosen so the kernel does ZERO transposes: the host ships x
pre-transposed (feature-major [E, S] bf16) and weights pre-chunked into their
SBUF layouts; the kernel emits the output feature-major bf16 and the host
transposes/casts it back.

Per-core pipeline (feature-major activations, bf16 matmuls, f32 PSUM):
  phase A: K,V token-major; VKT[m,d] (block-diag per head) and Ksum
           accumulated in PSUM over all S.  V bias folded into the psum->sbuf
           copy; K bias via a contraction-1 matmul.
  bridge:  WoKV[c,co] = VKT_c^T @ Wo_block — folds the output projection into
           the attention matmul (attn@Wo == (Q*Z) @ WoKV).
  phase B: software-pipelined over token tiles; Q = elu(Wq^T xT + bq)+1
           computed just-in-time per tile.  Z denominators for all 4 chunks
           packed at 32-aligned partitions of one PSUM tile -> one
           reciprocal_approx_fast per tile.  LN stats share one PSUM bank at
           partitions 0/32; rsqrt via recip_approx + Sqrt.
"""

import numpy as np
import ml_dtypes

from concourse import bass, bacc, tile, mybir
from concourse.bass_utils import run_bass_kernel_spmd

BF16 = ml_dtypes.bfloat16
F32 = np.float32

B, S, E, NH, HID, DH = 8, 4096, 512, 8, 2048, 64
ATTN_EPS = 1e-6
LN_EPS = 1e-5

NCORES = 8
TT = 512                  # tokens per tile
NT = S // TT              # 8 token tiles
NC_E = E // 128           # 4 feature chunks
NC_H = HID // 128         # 16 hidden chunks
NJ = TT // 128            # 4 token sub-tiles per tile

dt = mybir.dt
AF = mybir.ActivationFunctionType
ALU = mybir.AluOpType

_CACHE = {}


def _build():
    nc = bacc.Bacc("TRN2", target_bir_lowering=False, debug=False,
                   num_devices=NCORES)

    def din(name, shape, d):
        return nc.dram_tensor(name, list(shape), d, kind="ExternalInput")

    xt_d = din("xt", (E, S), dt.bfloat16)
    wq_d = din("wq", (128, NC_E * E), dt.bfloat16)
    wk_d = din("wk", (128, NC_E * E), dt.bfloat16)
    wv_d = din("wv", (128, NC_E * E), dt.bfloat16)
    wo_d = din("wo", (128, NC_E * E), dt.bfloat16)
    w1_d = din("w1", (128, NC_E * HID), dt.bfloat16)
    w2_d = din("w2", (128, NC_H * E), dt.bfloat16)
    pp_d = din("pp", (128, 44), dt.float32)
    aux_d = din("aux", (128, 3), dt.bfloat16)     # hsel (2 cols), ones col
    hexp4_d = din("hexp4", (128, 128), dt.bfloat16)
    onesr_d = din("onesr", (1, TT), dt.bfloat16)
    bkv_d = din("bkv", (2, E), dt.bfloat16)
    out_d = nc.dram_tensor("out", [E, S], dt.bfloat16, kind="ExternalOutput")

    with tile.TileContext(nc) as tc:
        from contextlib import ExitStack
        es = ExitStack()
        with es:
            cpool = es.enter_context(tc.tile_pool(name="const", bufs=1))

            wq_s = cpool.tile([128, NC_E * E], dt.bfloat16, tag="wq")
            wk_s = cpool.tile([128, NC_E * E], dt.bfloat16, tag="wk")
            wv_s = cpool.tile([128, NC_E * E], dt.bfloat16, tag="wv")
            wo_s = cpool.tile([128, NC_E * E], dt.bfloat16, tag="wo")
            w1_s = cpool.tile([128, NC_E * HID], dt.bfloat16, tag="w1")
            w2_s = cpool.tile([128, NC_H * E], dt.bfloat16, tag="w2")
            pp_s = cpool.tile([128, 44], dt.float32, tag="pp")
            aux_s = cpool.tile([128, 3], dt.bfloat16, tag="aux")
            hexp4_s = cpool.tile([128, 128], dt.bfloat16, tag="hexp4")
            onesr_s = cpool.tile([1, TT], dt.bfloat16, tag="onesr")
            bk_s = cpool.tile([1, E], dt.bfloat16, tag="bk")
            bv_s = cpool.tile([1, E], dt.bfloat16, tag="bv")
            xt_s = [cpool.tile([128, S], dt.bfloat16, tag=f"xt{c}", name=f"xt{c}")
                    for c in range(NC_E)]
            vkt_s = cpool.tile([128, NC_E * 128], dt.bfloat16, tag="vkt")
            wokv_s = cpool.tile([128, NC_E * NC_E * 128], dt.bfloat16,
                                tag="wokv")
            bvb_s = cpool.tile([128, E], dt.bfloat16, tag="bvb")
            ksumb_s = cpool.tile([1, E], dt.bfloat16, tag="ksumb")
            ksc_s = cpool.tile([128, NC_E], dt.float32, tag="ksc")
            ksel_s = cpool.tile([128, 2 * NC_E], dt.bfloat16, tag="ksel")

            # DMA issue order: x tiles 0-1 + Wk/Wv first (phase A), rest
            # later.  Weights on the Activation-engine HWDGE queue, x on the
            # sync queue, in parallel.
            for t in range(2):
                t0 = t * TT
                for c in range(NC_E):
                    nc.sync.dma_start(out=xt_s[c][:, t0:t0 + TT],
                                      in_=xt_d[c * 128:(c + 1) * 128,
                                               t0:t0 + TT])
            nc.scalar.dma_start(out=wk_s[:], in_=wk_d[:, :])
            nc.scalar.dma_start(out=wv_s[:], in_=wv_d[:, :])
            nc.scalar.dma_start(out=pp_s[:], in_=pp_d[:, :])
            nc.scalar.dma_start(out=aux_s[:], in_=aux_d[:, :])
            nc.scalar.dma_start(out=hexp4_s[:], in_=hexp4_d[:, :])
            nc.scalar.dma_start(out=onesr_s[:], in_=onesr_d[:, :])
            nc.scalar.dma_start(out=bk_s[:], in_=bkv_d[0:1, :])
            nc.scalar.dma_start(out=bv_s[:], in_=bkv_d[1:2, :])
            for c in range(NC_E):
                nc.sync.dma_start(out=xt_s[c][:, 2 * TT:],
                                  in_=xt_d[c * 128:(c + 1) * 128, 2 * TT:])
            nc.scalar.dma_start(out=wq_s[:], in_=wq_d[:, :])
            nc.scalar.dma_start(out=wo_s[:], in_=wo_d[:, :])
            nc.scalar.dma_start(out=w1_s[:], in_=w1_d[:, :])
            nc.scalar.dma_start(out=w2_s[:], in_=w2_d[:, :])

            hsel = aux_s[:, 0:2]             # [128,2] head select
            onesc = aux_s[:, 2:3]            # [128,1] ones col
            ones1x128 = onesr_s[0:1, 0:128]  # [1,128]
            bq_c = lambda c: pp_s[:, c:c + 1]
            bo_c = lambda c: pp_s[:, 4 + c:5 + c]
            b1_c = lambda j: pp_s[:, 8 + j:9 + j]
            b2_c = lambda c: pp_s[:, 24 + c:25 + c]
            g1_c = lambda c: pp_s[:, 28 + c:29 + c]
            be1_c = lambda c: pp_s[:, 32 + c:33 + c]
            g2_c = lambda c: pp_s[:, 36 + c:37 + c]
            be2_c = lambda c: pp_s[:, 40 + c:41 + c]

            # =========================== PHASE A ==========================
            # K,V token-major; accumulate VKT (block-diag) and Ksum.
            with tc.tile_pool(name="acc_ps", bufs=1, space="PSUM") as accp, \
                 tc.tile_pool(name="pa_ps", bufs=3, space="PSUM") as paps, \
                 tc.tile_pool(name="pa_t", bufs=3, space="SBUF") as pat, \
                 tc.tile_pool(name="pa_kv", bufs=3, space="SBUF") as pakv:

                vkt_ps = accp.tile([128, NC_E * 128], dt.float32, tag="vktp")
                ksum_ps = accp.tile([1, E], dt.float32, tag="ksump")
                # bvb = broadcast of bv over partitions (one-time)
                bvb_ps = paps.tile([128, E], dt.float32, tag="mm")
                nc.tensor.matmul(bvb_ps[:], ones1x128, bv_s[:],
                                 start=True, stop=True)
                nc.vector.tensor_copy(out=bvb_s[:], in_=bvb_ps[:])

                first_kv = True
                for t in range(NT):
                    t0 = t * TT
                    for j in range(NJ):
                        kps = paps.tile([128, E], dt.float32, tag="mm")
                        nc.tensor.matmul(kps[:], ones1x128, bk_s[:],
                                         start=True, stop=False,
                                         skip_group_check=True)
                        for ci in range(NC_E):
                            nc.tensor.matmul(
                                kps[:],
                                xt_s[ci][:, t0 + j * 128: t0 + (j + 1) * 128],
                                wk_s[:, ci * E:(ci + 1) * E],
                                start=False, stop=(ci == NC_E - 1),
                                skip_group_check=True)
                        kt = pakv.tile([128, E], dt.bfloat16, tag="kt")
                        t1 = pat.tile([128, E], dt.bfloat16, tag="t1")
                        nc.vector.tensor_scalar_max(t1[:], kps[:], 0.0)
                        nc.vector.tensor_scalar_min(kt[:], kps[:], 0.0)
                        nc.scalar.activation(kt[:], kt[:], AF.Exp)
                        nc.vector.tensor_add(kt[:], kt[:], t1[:])

                        vps = paps.tile([128, E], dt.float32, tag="mm")
                        for ci in range(NC_E):
                            nc.tensor.matmul(
                                vps[:],
                                xt_s[ci][:, t0 + j * 128: t0 + (j + 1) * 128],
                                wv_s[:, ci * E:(ci + 1) * E],
                                start=(ci == 0), stop=(ci == NC_E - 1),
                                skip_group_check=True)
                        vt = pakv.tile([128, E], dt.bfloat16, tag="vt")
                        nc.vector.tensor_add(vt[:], vps[:], bvb_s[:])

                        last_kv = (t == NT - 1) and (j == NJ - 1)
                        for c in range(NC_E):
                            nc.tensor.matmul(
                                vkt_ps[:, c * 128:(c + 1) * 128],
                                vt[:, c * 128:(c + 1) * 128],
                                kt[:, c * 128:(c + 1) * 128],
                                start=first_kv, stop=last_kv,
                                skip_group_check=True)
                        nc.tensor.matmul(ksum_ps[:], onesc, kt[:],
                                         start=first_kv, stop=last_kv,
                                         skip_group_check=True)
                        first_kv = False

                # ---- extract blockdiag VKT and Ksum^T chunks ----
                nc.vector.memset(vkt_s[:], 0.0)
                for c in range(NC_E):
                    for h in range(2):
                        o = c * 128 + h * 64
                        nc.vector.tensor_copy(
                            out=vkt_s[h * 64:(h + 1) * 64, o:o + 64],
                            in_=vkt_ps[h * 64:(h + 1) * 64, o:o + 64])
                nc.scalar.activation(ksumb_s[:], ksum_ps[:], AF.Copy)
                for c in range(NC_E):
                    ps = paps.tile([128, 1], dt.float32, tag="tpks", bufs=1)
                    nc.tensor.matmul(ps[0:128, 0:1],
                                     ksumb_s[0:1, c * 128:(c + 1) * 128],
                                     onesr_s[0:1, 0:1],
                                     start=True, stop=True)
                    nc.vector.tensor_copy(out=ksc_s[:, c:c + 1],
                                          in_=ps[0:128, 0:1])
                for c in range(NC_E):
                    nc.vector.tensor_scalar_mul(
                        ksel_s[:, 2 * c:2 * c + 2], hsel,
                        ksc_s[:, c:c + 1])
                # ---- WoKV[c,co] = VKT_c^T @ Wo_block (one-time) ----
                for c in range(NC_E):
                    for co in range(NC_E):
                        wps = paps.tile([128, 128], dt.float32, tag="wokv",
                                        bufs=2)
                        nc.tensor.matmul(
                            wps[:], vkt_s[:, c * 128:(c + 1) * 128],
                            wo_s[:, c * E + co * 128: c * E + (co + 1) * 128],
                            start=True, stop=True)
                        nc.vector.tensor_copy(
                            out=wokv_s[:, (c * NC_E + co) * 128:
                                       (c * NC_E + co + 1) * 128],
                            in_=wps[:])

            # =========================== PHASE B ==========================
            with tc.tile_pool(name="pb_ps", bufs=3, space="PSUM") as pbps, \
                 tc.tile_pool(name="pb_bc", bufs=2, space="PSUM") as pbbc, \
                 tc.tile_pool(name="pb_st", bufs=2, space="PSUM") as pbst, \
                 tc.tile_pool(name="pb_zd", bufs=1, space="PSUM") as pbzd, \
                 tc.tile_pool(name="pb_sb", bufs=3, space="SBUF") as pbsb, \
                 tc.tile_pool(name="pb_q", bufs=6, space="SBUF") as pbq, \
                 tc.tile_pool(name="pb_x1", bufs=6, space="SBUF") as pbx1, \
                 tc.tile_pool(name="pb_h", bufs=16, space="SBUF") as pbh, \
                 tc.tile_pool(name="pb_o", bufs=6, space="SBUF") as pbo:

                state = {}

                def ln_smalls(stat):
                    inv = 1.0 / E
                    mean = pbsb.tile([1, TT], dt.float32, tag="mean", bufs=2)
                    nc.vector.tensor_scalar_mul(mean[:], stat[0:1, :], inv)
                    msq = pbsb.tile([1, TT], dt.float32, tag="msq", bufs=2)
                    nc.vector.tensor_mul(msq[:], mean[:], mean[:])
                    var = pbsb.tile([1, TT], dt.float32, tag="var", bufs=2)
                    nc.vector.scalar_tensor_tensor(
                        out=var[:], in0=stat[32:33, :], scalar=inv,
                        in1=msq[:], op0=ALU.mult, op1=ALU.subtract)
                    rsf = pbsb.tile([1, TT], dt.float32, tag="rsf", bufs=2)
                    nc.vector.reciprocal_approx_fast(out=rsf[:], in_=var[:])
                    rs_b = pbsb.tile([1, TT], dt.bfloat16, tag="rsb", bufs=2)
                    nc.scalar.activation(rs_b[:], rsf[:], AF.Sqrt)
                    mean_b = pbsb.tile([1, TT], dt.bfloat16, tag="meanb",
                                       bufs=2)
                    nc.scalar.activation(mean_b[:], mean[:], AF.Copy)
                    return mean_b, rs_b

                def ln_finish(mean_b, rs_b, hts, g_c, be_c, opool, otag):
                    mb = pbbc.tile([128, TT], dt.float32, tag="bc")
                    nc.tensor.matmul(mb[:], ones1x128, mean_b[:],
                                     start=True, stop=True)
                    rb = pbbc.tile([128, TT], dt.float32, tag="bc")
                    nc.tensor.matmul(rb[:], ones1x128, rs_b[:],
                                     start=True, stop=True)
                    outs = []
                    for c in range(len(hts)):
                        tmp = pbsb.tile([128, TT], dt.bfloat16, tag="nrm")
                        nc.vector.tensor_sub(tmp[:], hts[c][:], mb[:])
                        nc.vector.tensor_mul(tmp[:], tmp[:], rb[:])
                        o = opool.tile([128, TT], dt.bfloat16, tag=otag)
                        nc.scalar.activation(o[:], tmp[:], AF.Identity,
                                             bias=be_c(c), scale=g_c(c))
                        outs.append(o)
                    return outs

                def s1_attn_wo(t):
                    t0 = t * TT
                    # -- Q = elu(Wq^T xT + bq)+1, just-in-time --
                    qts = []
                    for co in range(NC_E):
                        qps = pbps.tile([128, TT], dt.float32, tag="mm")
                        for ci in range(NC_E):
                            nc.tensor.matmul(
                                qps[:],
                                wq_s[:, ci * E + co * 128:
                                     ci * E + (co + 1) * 128],
                                xt_s[ci][:, t0:t0 + TT],
                                start=(ci == 0), stop=(ci == NC_E - 1))
                        t1 = pbsb.tile([128, TT], dt.bfloat16, tag="t1")
                        qt = pbq.tile([128, TT], dt.bfloat16, tag="qt")
                        nc.scalar.activation(t1[:], qps[:], AF.Relu,
                                             bias=bq_c(co))
                        nc.vector.tensor_scalar(
                            out=qt[:], in0=qps[:], scalar1=bq_c(co),
                            scalar2=0.0, op0=ALU.add, op1=ALU.min)
                        nc.scalar.activation(qt[:], qt[:], AF.Exp)
                        nc.vector.tensor_add(qt[:], qt[:], t1[:])
                        qts.append(qt)
                    # -- Z denominators, packed; one fast reciprocal --
                    zden = pbzd.tile([128, TT], dt.float32, tag="zd")
                    for c in range(NC_E):
                        nc.tensor.matmul(zden[32 * c:32 * c + 2, :],
                                         ksel_s[:, 2 * c:2 * c + 2],
                                         qts[c][:],
                                         start=True, stop=True,
                                         skip_group_check=True,
                                         tile_position=(0, 32 * c))
                    zrf = pbsb.tile([128, TT], dt.float32, tag="zrf", bufs=2)
                    nc.vector.reciprocal_approx_fast(out=zrf[:], in_=zden[:])
                    zrb = pbsb.tile([128, TT], dt.bfloat16, tag="zrb", bufs=2)
                    nc.scalar.activation(zrb[:], zrf[:], AF.Copy)
                    # -- qz = Q * Z (broadcast via hexp4 matmul) --
                    qzs = []
                    for c in range(NC_E):
                        zb = pbbc.tile([128, TT], dt.float32, tag="bc")
                        nc.tensor.matmul(zb[:],
                                         hexp4_s[32 * c:32 * c + 2, :],
                                         zrb[32 * c:32 * c + 2, :],
                                         start=True, stop=True,
                                         tile_position=(32 * c, 0))
                        qz = pbq.tile([128, TT], dt.bfloat16, tag="qz")
                        nc.vector.tensor_mul(qz[:], qts[c][:], zb[:])
                        qzs.append(qz)
                    # -- attn @ Wo fused via WoKV + residual + LN1 stats --
                    h1ts = []
                    stat1 = pbst.tile([128, TT], dt.float32, tag="st")
                    for co in range(NC_E):
                        ops_ = pbps.tile([128, TT], dt.float32, tag="mm")
                        for c in range(NC_E):
                            nc.tensor.matmul(
                                ops_[:],
                                wokv_s[:, (c * NC_E + co) * 128:
                                       (c * NC_E + co + 1) * 128],
                                qzs[c][:],
                                start=(c == 0), stop=(c == NC_E - 1))
                        h1t = pbx1.tile([128, TT], dt.bfloat16, tag="h1",
                                        bufs=4)
                        nc.vector.scalar_tensor_tensor(
                            out=h1t[:], in0=ops_[:], scalar=bo_c(co),
                            in1=xt_s[co][:, t0:t0 + TT],
                            op0=ALU.add, op1=ALU.add)
                        h1ts.append(h1t)
                        sq = pbsb.tile([128, TT], dt.bfloat16, tag="sq",
                                       bufs=4)
                        nc.vector.tensor_mul(sq[:], h1t[:], h1t[:])
                        nc.tensor.matmul(stat1[0:1, :], onesc, h1t[:],
                                         start=(co == 0),
                                         stop=(co == NC_E - 1),
                                         skip_group_check=True,
                                         tile_position=(0, 0))
                        nc.tensor.matmul(stat1[32:33, :], onesc, sq[:],
                                         start=(co == 0),
                                         stop=(co == NC_E - 1),
                                         skip_group_check=True,
                                         tile_position=(0, 32))
                    mb1, rb1 = ln_smalls(stat1)
                    state[t] = dict(h1ts=h1ts, mb1=mb1, rb1=rb1)

                def s2_ln1(t):
                    st = state[t]
                    st["x1ts"] = ln_finish(st["mb1"], st["rb1"], st["h1ts"],
                                           g1_c, be1_c, pbx1, "x1")

                def s3_ffn1(t):
                    x1ts = state[t]["x1ts"]
                    hts = []
                    for j in range(NC_H):
                        hps = pbps.tile([128, TT], dt.float32, tag="mm")
                        for ci in range(NC_E):
                            nc.tensor.matmul(
                                hps[:],
                                w1_s[:, ci * HID + j * 128:
                                     ci * HID + (j + 1) * 128],
                                x1ts[ci][:],
                                start=(ci == 0), stop=(ci == NC_E - 1))
                        ht = pbh.tile([128, TT], dt.bfloat16, tag="ht")
                        nc.scalar.activation(ht[:], hps[:], AF.Relu,
                                             bias=b1_c(j))
                        hts.append(ht)
                    state[t]["hts"] = hts

                def s4_ffn2(t):
                    x1ts = state[t]["x1ts"]
                    hts = state[t]["hts"]
                    h2ts = []
                    stat2 = pbst.tile([128, TT], dt.float32, tag="st")
                    for co in range(NC_E):
                        ops2 = pbps.tile([128, TT], dt.float32, tag="mm")
                        for j in range(NC_H):
                            nc.tensor.matmul(
                                ops2[:],
                                w2_s[:, j * E + co * 128:
                                     j * E + (co + 1) * 128],
                                hts[j][:],
                                start=(j == 0), stop=(j == NC_H - 1))
                        h2t = pbo.tile([128, TT], dt.bfloat16, tag="h2")
                        nc.vector.scalar_tensor_tensor(
                            out=h2t[:], in0=ops2[:], scalar=b2_c(co),
                            in1=x1ts[co][:], op0=ALU.add, op1=ALU.add)
                        h2ts.append(h2t)
                        sq = pbsb.tile([128, TT], dt.bfloat16, tag="sq",
                                       bufs=4)
                        nc.vector.tensor_mul(sq[:], h2t[:], h2t[:])
                        nc.tensor.matmul(stat2[0:1, :], onesc, h2t[:],
                                         start=(co == 0),
                                         stop=(co == NC_E - 1),
                                         skip_group_check=True,
                                         tile_position=(0, 0))
                        nc.tensor.matmul(stat2[32:33, :], onesc, sq[:],
                                         start=(co == 0),
                                         stop=(co == NC_E - 1),
                                         skip_group_check=True,
                                         tile_position=(0, 32))
                    mb2, rb2 = ln_smalls(stat2)
                    state[t].update(h2ts=h2ts, mb2=mb2, rb2=rb2)

                def s5_out(t):
                    t0 = t * TT
                    st = state[t]
                    outs = ln_finish(st["mb2"], st["rb2"], st["h2ts"],
                                     g2_c, be2_c, pbo, "ou")
                    for c in range(NC_E):
                        nc.sync.dma_start(
                            out=out_d[c * 128:(c + 1) * 128, t0:t0 + TT],
                            in_=outs[c][:])
                    del state[t]

                # software pipeline: FFN(t) overlaps attention/LN1(t+1)
                s1_attn_wo(0)
                s2_ln1(0)
                for t in range(NT):
                    s3_ffn1(t)
                    if t + 1 < NT:
                        s1_attn_wo(t + 1)
                    s4_ffn2(t)
                    if t + 1 < NT:
                        s2_ln1(t + 1)
                    s5_out(t)

    nc.compile()
    return nc


def _aux_arrays():
    aux = np.zeros((128, 3), dtype=BF16)
    aux[0:64, 0] = BF16(1.0)
    aux[64:128, 1] = BF16(1.0)
    aux[:, 2] = BF16(1.0)
    hexp4 = np.zeros((128, 128), dtype=BF16)
    for c in range(4):
        hexp4[32 * c, 0:64] = BF16(1.0)
        hexp4[32 * c + 1, 64:128] = BF16(1.0)
    onesr = np.ones((1, TT), dtype=BF16)
    return aux, hexp4, onesr


def _chunk_weight(w, nchunks):
    """[nchunks*128, X] f32 -> [128, nchunks*X] bf16 in c-major free layout."""
    X = w.shape[1]
    return np.ascontiguousarray(
        w.reshape(nchunks, 128, X).transpose(1, 0, 2).reshape(128, nchunks * X)
    ).astype(BF16)


def kernel(**inputs):
    if "nc" not in _CACHE:
        _CACHE["nc"] = _build()
    nc = _CACHE["nc"]

    x = np.asarray(inputs["x"], dtype=F32)
    aux, hexp4, onesr = _aux_arrays()
    pp = np.zeros((128, 44), dtype=F32)
    for c in range(4):
        pp[:, c] = inputs["bq"][c * 128:(c + 1) * 128]
        pp[:, 4 + c] = inputs["bo"][c * 128:(c + 1) * 128]
        pp[:, 24 + c] = inputs["b2"][c * 128:(c + 1) * 128]
        pp[:, 28 + c] = inputs["g1"][c * 128:(c + 1) * 128]
        pp[:, 32 + c] = inputs["be1"][c * 128:(c + 1) * 128]
        pp[:, 36 + c] = inputs["g2"][c * 128:(c + 1) * 128]
        pp[:, 40 + c] = inputs["be2"][c * 128:(c + 1) * 128]
    for j in range(16):
        pp[:, 8 + j] = inputs["b1"][j * 128:(j + 1) * 128]
    bkv = np.stack([np.asarray(inputs["bk"], F32),
                    np.asarray(inputs["bv"], F32)]).astype(BF16)

    shared = {
        "wq": _chunk_weight(np.asarray(inputs["Wq"], F32), NC_E),
        "wk": _chunk_weight(np.asarray(inputs["Wk"], F32), NC_E),
        "wv": _chunk_weight(np.asarray(inputs["Wv"], F32), NC_E),
        "wo": _chunk_weight(np.asarray(inputs["Wo"], F32), NC_E),
        "w1": _chunk_weight(np.asarray(inputs["W1"], F32), NC_E),
        "w2": _chunk_weight(np.asarray(inputs["W2"], F32), NC_H),
        "pp": pp, "aux": aux, "hexp4": hexp4, "onesr": onesr,
        "bkv": bkv,
    }
    in_maps = []
    for b in range(NCORES):
        m = dict(shared)
        m["xt"] = np.ascontiguousarray(x[b].T).astype(BF16)
        in_maps.append(m)

    res = run_bass_kernel_spmd(nc, in_maps, core_ids=list(range(NCORES)),
                               **_CACHE.get("run_kwargs", {}))
    _CACHE["last"] = res
    outs = [np.asarray(res.results[b]["out"]).astype(F32).T
            for b in range(NCORES)]
    return np.stack(outs, axis=0)


# revision 23
# speedup vs baseline: 1.1015x; 1.1015x over previous
"""Trainium2 Bass kernel for a linear-attention transformer block.

B=8, S=4096, E=512, NH=8, DH=64, HID=2048.
Sharding: data-parallel over batch — one batch element per NeuronCore, all
weights replicated, zero collectives.

Layouts are chosen so the kernel does ZERO transposes: the host ships x
pre-transposed (feature-major [E, S] bf16) and weights pre-chunked into their
SBUF layouts; the kernel emits the output feature-major bf16 and the host
transposes/casts it back.

Per-core pipeline (feature-major activations, bf16 matmuls, f32 PSUM):
  phase A: K,V token-major; VKT[m,d] (block-diag per head) and Ksum
           accumulated in PSUM over all S.  V bias folded into the psum->sbuf
           copy; K bias via a contraction-1 matmul.
  bridge:  WoKV[c,co] = VKT_c^T @ Wo_block — folds the output projection into
           the attention matmul (attn@Wo == (Q*Z) @ WoKV).
  phase B: software-pipelined over token tiles; Q = elu(Wq^T xT + bq)+1
           computed just-in-time per tile.  Z denominators for all 4 chunks
           packed at 32-aligned partitions of one PSUM tile -> one
           reciprocal_approx_fast per tile.  LN stats share one PSUM bank at
           partitions 0/32; rsqrt via recip_approx + Sqrt.
"""

import numpy as np
import ml_dtypes

from concourse import bass, bacc, tile, mybir
from concourse.bass_utils import run_bass_kernel_spmd

BF16 = ml_dtypes.bfloat16
F32 = np.float32

B, S, E, NH, HID, DH = 8, 4096, 512, 8, 2048, 64
ATTN_EPS = 1e-6
LN_EPS = 1e-5

NCORES = 8
TT = 512                  # tokens per tile
NT = S // TT              # 8 token tiles
NC_E = E // 128           # 4 feature chunks
NC_H = HID // 128         # 16 hidden chunks
NJ = TT // 128            # 4 token sub-tiles per tile

dt = mybir.dt
AF = mybir.ActivationFunctionType
ALU = mybir.AluOpType

_CACHE = {}


def _build():
    nc = bacc.Bacc("TRN2", target_bir_lowering=False, debug=False,
                   num_devices=NCORES)

    def din(name, shape, d):
        return nc.dram_tensor(name, list(shape), d, kind="ExternalInput")

    xt_d = din("xt", (E, S), dt.bfloat16)
    wq_d = din("wq", (128, NC_E * E), dt.bfloat16)
    wk_d = din("wk", (128, NC_E * E), dt.bfloat16)
    wv_d = din("wv", (128, NC_E * E), dt.bfloat16)
    wo_d = din("wo", (128, NC_E * E), dt.bfloat16)
    w1_d = din("w1", (128, NC_E * HID), dt.bfloat16)
    w2_d = din("w2", (128, NC_H * E), dt.bfloat16)
    pp_d = din("pp", (128, 44), dt.float32)
    aux_d = din("aux", (128, 3), dt.bfloat16)     # hsel (2 cols), ones col
    hexp4_d = din("hexp4", (128, 128), dt.bfloat16)
    onesr_d = din("onesr", (1, TT), dt.bfloat16)
    bkv_d = din("bkv", (2, E), dt.bfloat16)
    out_d = nc.dram_tensor("out", [E, S], dt.bfloat16, kind="ExternalOutput")

    with tile.TileContext(nc) as tc:
        from contextlib import ExitStack
        es = ExitStack()
        with es:
            cpool = es.enter_context(tc.tile_pool(name="const", bufs=1))

            wq_s = cpool.tile([128, NC_E * E], dt.bfloat16, tag="wq")
            wk_s = cpool.tile([128, NC_E * E], dt.bfloat16, tag="wk")
            wv_s = cpool.tile([128, NC_E * E], dt.bfloat16, tag="wv")
            wo_s = cpool.tile([128, NC_E * E], dt.bfloat16, tag="wo")
            w1_s = cpool.tile([128, NC_E * HID], dt.bfloat16, tag="w1")
            w2_s = cpool.tile([128, NC_H * E], dt.bfloat16, tag="w2")
            pp_s = cpool.tile([128, 44], dt.float32, tag="pp")
            aux_s = cpool.tile([128, 3], dt.bfloat16, tag="aux")
            hexp4_s = cpool.tile([128, 128], dt.bfloat16, tag="hexp4")
            onesr_s = cpool.tile([1, TT], dt.bfloat16, tag="onesr")
            bk_s = cpool.tile([1, E], dt.bfloat16, tag="bk")
            bv_s = cpool.tile([1, E], dt.bfloat16, tag="bv")
            xt_s = [cpool.tile([128, S], dt.bfloat16, tag=f"xt{c}", name=f"xt{c}")
                    for c in range(NC_E)]
            vkt_s = cpool.tile([128, NC_E * 128], dt.bfloat16, tag="vkt")
            wokv_s = cpool.tile([128, NC_E * NC_E * 128], dt.bfloat16,
                                tag="wokv")
            bvb_s = cpool.tile([128, E], dt.bfloat16, tag="bvb")
            ksumb_s = cpool.tile([1, E], dt.bfloat16, tag="ksumb")
            ksc_s = cpool.tile([128, NC_E], dt.float32, tag="ksc")
            ksel_s = cpool.tile([128, 2 * NC_E], dt.bfloat16, tag="ksel")

            # DMA issue order: x tiles 0-1 + Wk/Wv first (phase A), rest
            # later.  Weights on the Activation-engine HWDGE queue, x on the
            # sync queue, in parallel.
            for t in range(2):
                t0 = t * TT
                for c in range(NC_E):
                    nc.sync.dma_start(out=xt_s[c][:, t0:t0 + TT],
                                      in_=xt_d[c * 128:(c + 1) * 128,
                                               t0:t0 + TT])
            nc.scalar.dma_start(out=wk_s[:], in_=wk_d[:, :])
            nc.scalar.dma_start(out=wv_s[:], in_=wv_d[:, :])
            nc.scalar.dma_start(out=pp_s[:], in_=pp_d[:, :])
            nc.scalar.dma_start(out=aux_s[:], in_=aux_d[:, :])
            nc.scalar.dma_start(out=hexp4_s[:], in_=hexp4_d[:, :])
            nc.scalar.dma_start(out=onesr_s[:], in_=onesr_d[:, :])
            nc.scalar.dma_start(out=bk_s[:], in_=bkv_d[0:1, :])
            nc.scalar.dma_start(out=bv_s[:], in_=bkv_d[1:2, :])
            for c in range(NC_E):
                nc.sync.dma_start(out=xt_s[c][:, 2 * TT:],
                                  in_=xt_d[c * 128:(c + 1) * 128, 2 * TT:])
            nc.scalar.dma_start(out=wq_s[:], in_=wq_d[:, :])
            nc.scalar.dma_start(out=wo_s[:], in_=wo_d[:, :])
            nc.scalar.dma_start(out=w1_s[:], in_=w1_d[:, :])
            nc.scalar.dma_start(out=w2_s[:], in_=w2_d[:, :])

            hsel = aux_s[:, 0:2]             # [128,2] head select
            onesc = aux_s[:, 2:3]            # [128,1] ones col
            ones1x128 = onesr_s[0:1, 0:128]  # [1,128]
            bq_c = lambda c: pp_s[:, c:c + 1]
            bo_c = lambda c: pp_s[:, 4 + c:5 + c]
            b1_c = lambda j: pp_s[:, 8 + j:9 + j]
            b2_c = lambda c: pp_s[:, 24 + c:25 + c]
            g1_c = lambda c: pp_s[:, 28 + c:29 + c]
            be1_c = lambda c: pp_s[:, 32 + c:33 + c]
            g2_c = lambda c: pp_s[:, 36 + c:37 + c]
            be2_c = lambda c: pp_s[:, 40 + c:41 + c]

            # =========================== PHASE A ==========================
            # K,V token-major; accumulate VKT (block-diag) and Ksum.
            with tc.tile_pool(name="acc_ps", bufs=1, space="PSUM") as accp, \
                 tc.tile_pool(name="pa_ps", bufs=4, space="PSUM") as paps, \
                 tc.tile_pool(name="pa_t", bufs=4, space="SBUF") as pat, \
                 tc.tile_pool(name="pa_kv", bufs=4, space="SBUF") as pakv:

                vkt_ps = accp.tile([128, NC_E * 128], dt.float32, tag="vktp")
                ksum_ps = accp.tile([1, E], dt.float32, tag="ksump")
                # bvb = broadcast of bv over partitions (one-time)
                bvb_ps = paps.tile([128, E], dt.float32, tag="mm")
                nc.tensor.matmul(bvb_ps[:], ones1x128, bv_s[:],
                                 start=True, stop=True)
                nc.vector.tensor_copy(out=bvb_s[:], in_=bvb_ps[:])

                first_kv = True
                for t in range(NT):
                    t0 = t * TT
                    for j in range(NJ):
                        kps = paps.tile([128, E], dt.float32, tag="mm")
                        nc.tensor.matmul(kps[:], ones1x128, bk_s[:],
                                         start=True, stop=False,
                                         skip_group_check=True)
                        for ci in range(NC_E):
                            nc.tensor.matmul(
                                kps[:],
                                xt_s[ci][:, t0 + j * 128: t0 + (j + 1) * 128],
                                wk_s[:, ci * E:(ci + 1) * E],
                                start=False, stop=(ci == NC_E - 1),
                                skip_group_check=True)
                        kt = pakv.tile([128, E], dt.bfloat16, tag="kt")
                        t1 = pat.tile([128, E], dt.bfloat16, tag="t1")
                        nc.scalar.activation(t1[:], kps[:], AF.Relu)
                        nc.vector.tensor_scalar_min(kt[:], kps[:], 0.0)
                        nc.scalar.activation(kt[:], kt[:], AF.Exp)
                        nc.vector.tensor_add(kt[:], kt[:], t1[:])

                        vps = paps.tile([128, E], dt.float32, tag="mm")
                        for ci in range(NC_E):
                            nc.tensor.matmul(
                                vps[:],
                                xt_s[ci][:, t0 + j * 128: t0 + (j + 1) * 128],
                                wv_s[:, ci * E:(ci + 1) * E],
                                start=(ci == 0), stop=(ci == NC_E - 1),
                                skip_group_check=True)
                        vt = pakv.tile([128, E], dt.bfloat16, tag="vt")
                        nc.vector.tensor_add(vt[:], vps[:], bvb_s[:])

                        last_kv = (t == NT - 1) and (j == NJ - 1)
                        for c in range(NC_E):
                            nc.tensor.matmul(
                                vkt_ps[:, c * 128:(c + 1) * 128],
                                vt[:, c * 128:(c + 1) * 128],
                                kt[:, c * 128:(c + 1) * 128],
                                start=first_kv, stop=last_kv,
                                skip_group_check=True)
                        nc.tensor.matmul(ksum_ps[:], onesc, kt[:],
                                         start=first_kv, stop=last_kv,
                                         skip_group_check=True)
                        first_kv = False

                # ---- extract blockdiag VKT and Ksum^T chunks ----
                nc.vector.memset(vkt_s[:], 0.0)
                for c in range(NC_E):
                    for h in range(2):
                        o = c * 128 + h * 64
                        nc.vector.tensor_copy(
                            out=vkt_s[h * 64:(h + 1) * 64, o:o + 64],
                            in_=vkt_ps[h * 64:(h + 1) * 64, o:o + 64])
                nc.scalar.activation(ksumb_s[:], ksum_ps[:], AF.Copy)
                for c in range(NC_E):
                    ps = accp.tile([128, 1], dt.float32, tag="tpks", bufs=1)
                    nc.tensor.matmul(ps[0:128, 0:1],
                                     ksumb_s[0:1, c * 128:(c + 1) * 128],
                                     onesr_s[0:1, 0:1],
                                     start=True, stop=True)
                    nc.vector.tensor_copy(out=ksc_s[:, c:c + 1],
                                          in_=ps[0:128, 0:1])
                for c in range(NC_E):
                    nc.vector.tensor_scalar_mul(
                        ksel_s[:, 2 * c:2 * c + 2], hsel,
                        ksc_s[:, c:c + 1])
                # ---- WoKV[c,co] = VKT_c^T @ Wo_block (one-time) ----
                for c in range(NC_E):
                    for co in range(NC_E):
                        wps = accp.tile([128, 128], dt.float32, tag="wokv",
                                        bufs=1)
                        nc.tensor.matmul(
                            wps[:], vkt_s[:, c * 128:(c + 1) * 128],
                            wo_s[:, c * E + co * 128: c * E + (co + 1) * 128],
                            start=True, stop=True)
                        nc.vector.tensor_copy(
                            out=wokv_s[:, (c * NC_E + co) * 128:
                                       (c * NC_E + co + 1) * 128],
                            in_=wps[:])

            # =========================== PHASE B ==========================
            with tc.tile_pool(name="pb_ps", bufs=3, space="PSUM") as pbps, \
                 tc.tile_pool(name="pb_bc", bufs=2, space="PSUM") as pbbc, \
                 tc.tile_pool(name="pb_st", bufs=2, space="PSUM") as pbst, \
                 tc.tile_pool(name="pb_zd", bufs=1, space="PSUM") as pbzd, \
                 tc.tile_pool(name="pb_sb", bufs=3, space="SBUF") as pbsb, \
                 tc.tile_pool(name="pb_q", bufs=6, space="SBUF") as pbq, \
                 tc.tile_pool(name="pb_x1", bufs=6, space="SBUF") as pbx1, \
                 tc.tile_pool(name="pb_h", bufs=16, space="SBUF") as pbh, \
                 tc.tile_pool(name="pb_o", bufs=6, space="SBUF") as pbo:

                state = {}

                def ln_smalls(stat):
                    inv = 1.0 / E
                    mean = pbsb.tile([1, TT], dt.float32, tag="mean", bufs=2)
                    nc.vector.tensor_scalar_mul(mean[:], stat[0:1, :], inv)
                    msq = pbsb.tile([1, TT], dt.float32, tag="msq", bufs=2)
                    nc.vector.tensor_mul(msq[:], mean[:], mean[:])
                    var = pbsb.tile([1, TT], dt.float32, tag="var", bufs=2)
                    nc.vector.scalar_tensor_tensor(
                        out=var[:], in0=stat[32:33, :], scalar=inv,
                        in1=msq[:], op0=ALU.mult, op1=ALU.subtract)
                    rsf = pbsb.tile([1, TT], dt.float32, tag="rsf", bufs=2)
                    nc.vector.reciprocal_approx_fast(out=rsf[:], in_=var[:])
                    rs_b = pbsb.tile([1, TT], dt.bfloat16, tag="rsb", bufs=2)
                    nc.scalar.activation(rs_b[:], rsf[:], AF.Sqrt)
                    mean_b = pbsb.tile([1, TT], dt.bfloat16, tag="meanb",
                                       bufs=2)
                    nc.scalar.activation(mean_b[:], mean[:], AF.Copy)
                    return mean_b, rs_b

                def ln_finish(mean_b, rs_b, hts, g_c, be_c, opool, otag):
                    mbp = pbbc.tile([128, TT], dt.float32, tag="bc")
                    nc.tensor.matmul(mbp[:], ones1x128, mean_b[:],
                                     start=True, stop=True)
                    rbp = pbbc.tile([128, TT], dt.float32, tag="bc")
                    nc.tensor.matmul(rbp[:], ones1x128, rs_b[:],
                                     start=True, stop=True)
                    # bf16 SBUF copies so the per-chunk sub/mul get 2x DVE
                    mb = pbsb.tile([128, TT], dt.bfloat16, tag="mbb", bufs=2)
                    nc.scalar.activation(mb[:], mbp[:], AF.Copy)
                    rb = pbsb.tile([128, TT], dt.bfloat16, tag="rbb", bufs=2)
                    nc.scalar.activation(rb[:], rbp[:], AF.Copy)
                    outs = []
                    for c in range(len(hts)):
                        tmp = pbsb.tile([128, TT], dt.bfloat16, tag="nrm")
                        nc.vector.tensor_sub(tmp[:], hts[c][:], mb[:])
                        nc.vector.tensor_mul(tmp[:], tmp[:], rb[:])
                        o = opool.tile([128, TT], dt.bfloat16, tag=otag)
                        nc.scalar.activation(o[:], tmp[:], AF.Identity,
                                             bias=be_c(c), scale=g_c(c))
                        outs.append(o)
                    return outs

                def s1q_qproj(t):
                    t0 = t * TT
                    # -- Q = elu(Wq^T xT + bq)+1, just-in-time --
                    qts = []
                    for co in range(NC_E):
                        qps = pbps.tile([128, TT], dt.float32, tag="mm")
                        for ci in range(NC_E):
                            nc.tensor.matmul(
                                qps[:],
                                wq_s[:, ci * E + co * 128:
                                     ci * E + (co + 1) * 128],
                                xt_s[ci][:, t0:t0 + TT],
                                start=(ci == 0), stop=(ci == NC_E - 1))
                        t1 = pbsb.tile([128, TT], dt.bfloat16, tag="t1")
                        qt = pbq.tile([128, TT], dt.bfloat16, tag="qt",
                                      bufs=12)
                        nc.scalar.activation(t1[:], qps[:], AF.Relu,
                                             bias=bq_c(co))
                        nc.vector.tensor_scalar(
                            out=qt[:], in0=qps[:], scalar1=bq_c(co),
                            scalar2=0.0, op0=ALU.add, op1=ALU.min)
                        nc.scalar.activation(qt[:], qt[:], AF.Exp)
                        nc.vector.tensor_add(qt[:], qt[:], t1[:])
                        qts.append(qt)
                    state.setdefault(t, {})["qts"] = qts

                def s1z_attn_wo(t):
                    t0 = t * TT
                    qts = state[t]["qts"]
                    # -- Z denominators, packed; one fast reciprocal --
                    zden = pbzd.tile([128, TT], dt.float32, tag="zd")
                    for c in range(NC_E):
                        nc.tensor.matmul(zden[32 * c:32 * c + 2, :],
                                         ksel_s[:, 2 * c:2 * c + 2],
                                         qts[c][:],
                                         start=True, stop=True,
                                         skip_group_check=True,
                                         tile_position=(0, 32 * c))
                    zrf = pbsb.tile([128, TT], dt.float32, tag="zrf", bufs=2)
                    nc.vector.reciprocal_approx_fast(out=zrf[:], in_=zden[:])
                    zrb = pbsb.tile([128, TT], dt.bfloat16, tag="zrb", bufs=2)
                    nc.scalar.activation(zrb[:], zrf[:], AF.Copy)
                    # -- qz = Q * Z (broadcast via hexp4 matmul) --
                    qzs = []
                    for c in range(NC_E):
                        zb = pbbc.tile([128, TT], dt.float32, tag="bc")
                        nc.tensor.matmul(zb[:],
                                         hexp4_s[32 * c:32 * c + 2, :],
                                         zrb[32 * c:32 * c + 2, :],
                                         start=True, stop=True,
                                         tile_position=(32 * c, 0))
                        qz = pbq.tile([128, TT], dt.bfloat16, tag="qz")
                        nc.vector.tensor_mul(qz[:], qts[c][:], zb[:])
                        qzs.append(qz)
                    # -- attn @ Wo fused via WoKV + residual + LN1 stats --
                    h1ts = []
                    stat1 = pbst.tile([128, TT], dt.float32, tag="st")
                    for co in range(NC_E):
                        ops_ = pbps.tile([128, TT], dt.float32, tag="mm")
                        for c in range(NC_E):
                            nc.tensor.matmul(
                                ops_[:],
                                wokv_s[:, (c * NC_E + co) * 128:
                                       (c * NC_E + co + 1) * 128],
                                qzs[c][:],
                                start=(c == 0), stop=(c == NC_E - 1))
                        h1t = pbx1.tile([128, TT], dt.bfloat16, tag="h1",
                                        bufs=4)
                        nc.vector.scalar_tensor_tensor(
                            out=h1t[:], in0=ops_[:], scalar=bo_c(co),
                            in1=xt_s[co][:, t0:t0 + TT],
                            op0=ALU.add, op1=ALU.add)
                        h1ts.append(h1t)
                        sq = pbsb.tile([128, TT], dt.bfloat16, tag="sq",
                                       bufs=4)
                        nc.scalar.activation(sq[:], h1t[:], AF.Square)
                        nc.tensor.matmul(stat1[0:1, :], onesc, h1t[:],
                                         start=(co == 0),
                                         stop=(co == NC_E - 1),
                                         skip_group_check=True,
                                         tile_position=(0, 0))
                        nc.tensor.matmul(stat1[32:33, :], onesc, sq[:],
                                         start=(co == 0),
                                         stop=(co == NC_E - 1),
                                         skip_group_check=True,
                                         tile_position=(0, 32))
                    mb1, rb1 = ln_smalls(stat1)
                    state[t].update(h1ts=h1ts, mb1=mb1, rb1=rb1)

                def s2_ln1(t):
                    st = state[t]
                    st["x1ts"] = ln_finish(st["mb1"], st["rb1"], st["h1ts"],
                                           g1_c, be1_c, pbx1, "x1")

                def s3_ffn1(t):
                    x1ts = state[t]["x1ts"]
                    hts = []
                    for j in range(NC_H):
                        hps = pbps.tile([128, TT], dt.float32, tag="mm")
                        for ci in range(NC_E):
                            nc.tensor.matmul(
                                hps[:],
                                w1_s[:, ci * HID + j * 128:
                                     ci * HID + (j + 1) * 128],
                                x1ts[ci][:],
                                start=(ci == 0), stop=(ci == NC_E - 1))
                        ht = pbh.tile([128, TT], dt.bfloat16, tag="ht")
                        nc.scalar.activation(ht[:], hps[:], AF.Relu,
                                             bias=b1_c(j))
                        hts.append(ht)
                    state[t]["hts"] = hts

                def s4_ffn2(t):
                    x1ts = state[t]["x1ts"]
                    hts = state[t]["hts"]
                    h2ts = []
                    stat2 = pbst.tile([128, TT], dt.float32, tag="st")
                    for co in range(NC_E):
                        ops2 = pbps.tile([128, TT], dt.float32, tag="mm")
                        for j in range(NC_H):
                            nc.tensor.matmul(
                                ops2[:],
                                w2_s[:, j * E + co * 128:
                                     j * E + (co + 1) * 128],
                                hts[j][:],
                                start=(j == 0), stop=(j == NC_H - 1))
                        h2t = pbo.tile([128, TT], dt.bfloat16, tag="h2")
                        nc.vector.scalar_tensor_tensor(
                            out=h2t[:], in0=ops2[:], scalar=b2_c(co),
                            in1=x1ts[co][:], op0=ALU.add, op1=ALU.add)
                        h2ts.append(h2t)
                        sq = pbsb.tile([128, TT], dt.bfloat16, tag="sq",
                                       bufs=4)
                        nc.scalar.activation(sq[:], h2t[:], AF.Square)
                        nc.tensor.matmul(stat2[0:1, :], onesc, h2t[:],
                                         start=(co == 0),
                                         stop=(co == NC_E - 1),
                                         skip_group_check=True,
                                         tile_position=(0, 0))
                        nc.tensor.matmul(stat2[32:33, :], onesc, sq[:],
                                         start=(co == 0),
                                         stop=(co == NC_E - 1),
                                         skip_group_check=True,
                                         tile_position=(0, 32))
                    mb2, rb2 = ln_smalls(stat2)
                    state[t].update(h2ts=h2ts, mb2=mb2, rb2=rb2)

                def s5_out(t):
                    t0 = t * TT
                    st = state[t]
                    outs = ln_finish(st["mb2"], st["rb2"], st["h2ts"],
                                     g2_c, be2_c, pbo, "ou")
                    for c in range(NC_E):
                        nc.sync.dma_start(
                            out=out_d[c * 128:(c + 1) * 128, t0:t0 + TT],
                            in_=outs[c][:])
                    del state[t]

                # software pipeline, 3 tiles deep: FFN(t) overlaps
                # attention/LN1(t+1); Q-proj(t+2) fills the PE while the
                # x1(t+1) normalize chain completes.
                s1q_qproj(0)
                s1q_qproj(1)
                s1z_attn_wo(0)
                s2_ln1(0)
                for t in range(NT):
                    s3_ffn1(t)
                    if t + 1 < NT:
                        s1z_attn_wo(t + 1)
                    s4_ffn2(t)
                    if t + 1 < NT:
                        s2_ln1(t + 1)
                    if t + 2 < NT:
                        s1q_qproj(t + 2)
                    s5_out(t)

    nc.compile()
    return nc


def _aux_arrays():
    aux = np.zeros((128, 3), dtype=BF16)
    aux[0:64, 0] = BF16(1.0)
    aux[64:128, 1] = BF16(1.0)
    aux[:, 2] = BF16(1.0)
    hexp4 = np.zeros((128, 128), dtype=BF16)
    for c in range(4):
        hexp4[32 * c, 0:64] = BF16(1.0)
        hexp4[32 * c + 1, 64:128] = BF16(1.0)
    onesr = np.ones((1, TT), dtype=BF16)
    return aux, hexp4, onesr


def _chunk_weight(w, nchunks):
    """[nchunks*128, X] f32 -> [128, nchunks*X] bf16 in c-major free layout."""
    X = w.shape[1]
    return np.ascontiguousarray(
        w.reshape(nchunks, 128, X).transpose(1, 0, 2).reshape(128, nchunks * X)
    ).astype(BF16)


def kernel(**inputs):
    if "nc" not in _CACHE:
        _CACHE["nc"] = _build()
    nc = _CACHE["nc"]

    x = np.asarray(inputs["x"], dtype=F32)
    aux, hexp4, onesr = _aux_arrays()
    pp = np.zeros((128, 44), dtype=F32)
    for c in range(4):
        pp[:, c] = inputs["bq"][c * 128:(c + 1) * 128]
        pp[:, 4 + c] = inputs["bo"][c * 128:(c + 1) * 128]
        pp[:, 24 + c] = inputs["b2"][c * 128:(c + 1) * 128]
        pp[:, 28 + c] = inputs["g1"][c * 128:(c + 1) * 128]
        pp[:, 32 + c] = inputs["be1"][c * 128:(c + 1) * 128]
        pp[:, 36 + c] = inputs["g2"][c * 128:(c + 1) * 128]
        pp[:, 40 + c] = inputs["be2"][c * 128:(c + 1) * 128]
    for j in range(16):
        pp[:, 8 + j] = inputs["b1"][j * 128:(j + 1) * 128]
    bkv = np.stack([np.asarray(inputs["bk"], F32),
                    np.asarray(inputs["bv"], F32)]).astype(BF16)

    shared = {
        "wq": _chunk_weight(np.asarray(inputs["Wq"], F32), NC_E),
        "wk": _chunk_weight(np.asarray(inputs["Wk"], F32), NC_E),
        "wv": _chunk_weight(np.asarray(inputs["Wv"], F32), NC_E),
        "wo": _chunk_weight(np.asarray(inputs["Wo"], F32), NC_E),
        "w1": _chunk_weight(np.asarray(inputs["W1"], F32), NC_E),
        "w2": _chunk_weight(np.asarray(inputs["W2"], F32), NC_H),
        "pp": pp, "aux": aux, "hexp4": hexp4, "onesr": onesr,
        "bkv": bkv,
    }
    in_maps = []
    for b in range(NCORES):
        m = dict(shared)
        m["xt"] = np.ascontiguousarray(x[b].T).astype(BF16)
        in_maps.append(m)

    res = run_bass_kernel_spmd(nc, in_maps, core_ids=list(range(NCORES)),
                               **_CACHE.get("run_kwargs", {}))
    _CACHE["last"] = res
    outs = [np.asarray(res.results[b]["out"]).astype(F32).T
            for b in range(NCORES)]
    return np.stack(outs, axis=0)


# revision 27
# speedup vs baseline: 1.1034x; 1.0017x over previous
"""Trainium2 Bass kernel for a linear-attention transformer block.

B=8, S=4096, E=512, NH=8, DH=64, HID=2048.
Sharding: data-parallel over batch — one batch element per NeuronCore, all
weights replicated, zero collectives.

Layouts are chosen so the kernel does ZERO transposes: the host ships x
pre-transposed (feature-major [E, S] bf16) and weights pre-chunked into their
SBUF layouts; the kernel emits the output feature-major bf16 and the host
transposes/casts it back.

Per-core pipeline (feature-major activations, bf16 matmuls, f32 PSUM):
  phase A: K,V token-major; VKT[m,d] (block-diag per head) and Ksum
           accumulated in PSUM over all S.  V bias folded into the psum->sbuf
           copy; K bias via a contraction-1 matmul.
  bridge:  WoKV[c,co] = VKT_c^T @ Wo_block — folds the output projection into
           the attention matmul (attn@Wo == (Q*Z) @ WoKV).
  phase B: software-pipelined over token tiles; Q = elu(Wq^T xT + bq)+1
           computed just-in-time per tile.  Z denominators for all 4 chunks
           packed at 32-aligned partitions of one PSUM tile -> one
           reciprocal_approx_fast per tile.  LN stats share one PSUM bank at
           partitions 0/32; rsqrt via recip_approx + Sqrt.
"""

import numpy as np
import ml_dtypes

from concourse import bass, bacc, tile, mybir
from concourse.bass_utils import run_bass_kernel_spmd

BF16 = ml_dtypes.bfloat16
F32 = np.float32

B, S, E, NH, HID, DH = 8, 4096, 512, 8, 2048, 64
ATTN_EPS = 1e-6
LN_EPS = 1e-5

NCORES = 8
TT = 512                  # tokens per tile
NT = S // TT              # 8 token tiles
NC_E = E // 128           # 4 feature chunks
NC_H = HID // 128         # 16 hidden chunks
NJ = TT // 128            # 4 token sub-tiles per tile

dt = mybir.dt
AF = mybir.ActivationFunctionType
ALU = mybir.AluOpType

_CACHE = {}


def _build():
    nc = bacc.Bacc("TRN2", target_bir_lowering=False, debug=False,
                   num_devices=NCORES)

    def din(name, shape, d):
        return nc.dram_tensor(name, list(shape), d, kind="ExternalInput")

    xt_d = din("xt", (E, S), dt.bfloat16)
    wq_d = din("wq", (128, NC_E * E), dt.bfloat16)
    wk_d = din("wk", (128, NC_E * E), dt.bfloat16)
    wv_d = din("wv", (128, NC_E * E), dt.bfloat16)
    wo_d = din("wo", (128, NC_E * E), dt.bfloat16)
    w1_d = din("w1", (128, NC_E * HID), dt.bfloat16)
    w2_d = din("w2", (128, NC_H * E), dt.bfloat16)
    pp_d = din("pp", (128, 44), dt.float32)
    aux_d = din("aux", (128, 3), dt.bfloat16)     # hsel (2 cols), ones col
    hexp4_d = din("hexp4", (128, 128), dt.bfloat16)
    onesr_d = din("onesr", (1, TT), dt.bfloat16)
    bkv_d = din("bkv", (2, E), dt.bfloat16)
    out_d = nc.dram_tensor("out", [E, S], dt.bfloat16, kind="ExternalOutput")

    with tile.TileContext(nc) as tc:
        from contextlib import ExitStack
        es = ExitStack()
        with es:
            cpool = es.enter_context(tc.tile_pool(name="const", bufs=1))

            wq_s = cpool.tile([128, NC_E * E], dt.bfloat16, tag="wq")
            wk_s = cpool.tile([128, NC_E * E], dt.bfloat16, tag="wk")
            wv_s = cpool.tile([128, NC_E * E], dt.bfloat16, tag="wv")
            wo_s = cpool.tile([128, NC_E * E], dt.bfloat16, tag="wo")
            w1_s = cpool.tile([128, NC_E * HID], dt.bfloat16, tag="w1")
            w2_s = cpool.tile([128, NC_H * E], dt.bfloat16, tag="w2")
            pp_s = cpool.tile([128, 44], dt.float32, tag="pp")
            aux_s = cpool.tile([128, 3], dt.bfloat16, tag="aux")
            hexp4_s = cpool.tile([128, 128], dt.bfloat16, tag="hexp4")
            onesr_s = cpool.tile([1, TT], dt.bfloat16, tag="onesr")
            bk_s = cpool.tile([1, E], dt.bfloat16, tag="bk")
            bv_s = cpool.tile([1, E], dt.bfloat16, tag="bv")
            xt_s = [cpool.tile([128, S], dt.bfloat16, tag=f"xt{c}", name=f"xt{c}")
                    for c in range(NC_E)]
            vkt_s = cpool.tile([128, NC_E * 128], dt.bfloat16, tag="vkt")
            wokv_s = cpool.tile([128, NC_E * NC_E * 128], dt.bfloat16,
                                tag="wokv")
            bvb_s = cpool.tile([128, E], dt.bfloat16, tag="bvb")
            ksumb_s = cpool.tile([1, E], dt.bfloat16, tag="ksumb")
            ksc_s = cpool.tile([128, NC_E], dt.float32, tag="ksc")
            ksel_s = cpool.tile([128, 2 * NC_E], dt.bfloat16, tag="ksel")

            # DMA issue order: x tiles 0-1 + Wk/Wv first (phase A), rest
            # later.  Weights on the Activation-engine HWDGE queue, x on the
            # sync queue, in parallel.
            for t in range(2):
                t0 = t * TT
                for c in range(NC_E):
                    nc.sync.dma_start(out=xt_s[c][:, t0:t0 + TT],
                                      in_=xt_d[c * 128:(c + 1) * 128,
                                               t0:t0 + TT])
            nc.scalar.dma_start(out=wk_s[:], in_=wk_d[:, :])
            nc.scalar.dma_start(out=wv_s[:], in_=wv_d[:, :])
            nc.scalar.dma_start(out=pp_s[:], in_=pp_d[:, :])
            nc.scalar.dma_start(out=aux_s[:], in_=aux_d[:, :])
            nc.scalar.dma_start(out=hexp4_s[:], in_=hexp4_d[:, :])
            nc.scalar.dma_start(out=onesr_s[:], in_=onesr_d[:, :])
            nc.scalar.dma_start(out=bk_s[:], in_=bkv_d[0:1, :])
            nc.scalar.dma_start(out=bv_s[:], in_=bkv_d[1:2, :])
            for c in range(NC_E):
                nc.sync.dma_start(out=xt_s[c][:, 2 * TT:],
                                  in_=xt_d[c * 128:(c + 1) * 128, 2 * TT:])
            nc.scalar.dma_start(out=wq_s[:], in_=wq_d[:, :])
            nc.scalar.dma_start(out=wo_s[:], in_=wo_d[:, :])
            nc.scalar.dma_start(out=w1_s[:], in_=w1_d[:, :])
            nc.scalar.dma_start(out=w2_s[:], in_=w2_d[:, :])

            hsel = aux_s[:, 0:2]             # [128,2] head select
            onesc = aux_s[:, 2:3]            # [128,1] ones col
            ones1x128 = onesr_s[0:1, 0:128]  # [1,128]
            bq_c = lambda c: pp_s[:, c:c + 1]
            bo_c = lambda c: pp_s[:, 4 + c:5 + c]
            b1_c = lambda j: pp_s[:, 8 + j:9 + j]
            b2_c = lambda c: pp_s[:, 24 + c:25 + c]
            g1_c = lambda c: pp_s[:, 28 + c:29 + c]
            be1_c = lambda c: pp_s[:, 32 + c:33 + c]
            g2_c = lambda c: pp_s[:, 36 + c:37 + c]
            be2_c = lambda c: pp_s[:, 40 + c:41 + c]

            # =========================== PHASE A ==========================
            # K,V token-major; accumulate VKT (block-diag) and Ksum.
            with tc.tile_pool(name="acc_ps", bufs=1, space="PSUM") as accp, \
                 tc.tile_pool(name="pa_ps", bufs=4, space="PSUM") as paps, \
                 tc.tile_pool(name="pa_t", bufs=4, space="SBUF") as pat, \
                 tc.tile_pool(name="pa_kv", bufs=4, space="SBUF") as pakv:

                vkt_ps = accp.tile([128, NC_E * 128], dt.float32, tag="vktp")
                ksum_ps = accp.tile([1, E], dt.float32, tag="ksump")
                # bvb = broadcast of bv over partitions (one-time)
                bvb_ps = paps.tile([128, E], dt.float32, tag="mm")
                nc.tensor.matmul(bvb_ps[:], ones1x128, bv_s[:],
                                 start=True, stop=True)
                nc.vector.tensor_copy(out=bvb_s[:], in_=bvb_ps[:])

                # j-pipelined: VKT/Ksum accumulation of step i-1 is emitted
                # after the K/V projections of step i, so the PE never
                # head-of-line blocks on the elu chain.
                first_kv = True
                pending = None
                nsteps = NT * NJ

                def emit_acc(kt, vt, last_kv):
                    nonlocal first_kv
                    for c in range(NC_E):
                        nc.tensor.matmul(
                            vkt_ps[:, c * 128:(c + 1) * 128],
                            vt[:, c * 128:(c + 1) * 128],
                            kt[:, c * 128:(c + 1) * 128],
                            start=first_kv, stop=last_kv,
                            skip_group_check=True)
                    nc.tensor.matmul(ksum_ps[:], onesc, kt[:],
                                     start=first_kv, stop=last_kv,
                                     skip_group_check=True)
                    first_kv = False

                for step in range(nsteps):
                    t, j = divmod(step, NJ)
                    t0 = t * TT
                    kps = paps.tile([128, E], dt.float32, tag="mm")
                    nc.tensor.matmul(kps[:], ones1x128, bk_s[:],
                                     start=True, stop=False,
                                     skip_group_check=True)
                    for ci in range(NC_E):
                        nc.tensor.matmul(
                            kps[:],
                            xt_s[ci][:, t0 + j * 128: t0 + (j + 1) * 128],
                            wk_s[:, ci * E:(ci + 1) * E],
                            start=False, stop=(ci == NC_E - 1),
                            skip_group_check=True)
                    kt = pakv.tile([128, E], dt.bfloat16, tag="kt")
                    t1 = pat.tile([128, E], dt.bfloat16, tag="t1")
                    nc.scalar.activation(t1[:], kps[:], AF.Relu)
                    nc.vector.tensor_scalar_min(kt[:], kps[:], 0.0)
                    nc.scalar.activation(kt[:], kt[:], AF.Exp)
                    nc.vector.tensor_add(kt[:], kt[:], t1[:])

                    vps = paps.tile([128, E], dt.float32, tag="mm")
                    for ci in range(NC_E):
                        nc.tensor.matmul(
                            vps[:],
                            xt_s[ci][:, t0 + j * 128: t0 + (j + 1) * 128],
                            wv_s[:, ci * E:(ci + 1) * E],
                            start=(ci == 0), stop=(ci == NC_E - 1),
                            skip_group_check=True)
                    vt = pakv.tile([128, E], dt.bfloat16, tag="vt")
                    nc.vector.tensor_add(vt[:], vps[:], bvb_s[:])

                    if pending is not None:
                        emit_acc(*pending, False)
                    pending = (kt, vt)
                emit_acc(*pending, True)

                # ---- extract blockdiag VKT and Ksum^T chunks ----
                nc.vector.memset(vkt_s[:], 0.0)
                for c in range(NC_E):
                    for h in range(2):
                        o = c * 128 + h * 64
                        nc.vector.tensor_copy(
                            out=vkt_s[h * 64:(h + 1) * 64, o:o + 64],
                            in_=vkt_ps[h * 64:(h + 1) * 64, o:o + 64])
                nc.scalar.activation(ksumb_s[:], ksum_ps[:], AF.Copy)
                for c in range(NC_E):
                    ps = accp.tile([128, 1], dt.float32, tag="tpks", bufs=1)
                    nc.tensor.matmul(ps[0:128, 0:1],
                                     ksumb_s[0:1, c * 128:(c + 1) * 128],
                                     onesr_s[0:1, 0:1],
                                     start=True, stop=True)
                    nc.vector.tensor_copy(out=ksc_s[:, c:c + 1],
                                          in_=ps[0:128, 0:1])
                for c in range(NC_E):
                    nc.vector.tensor_scalar_mul(
                        ksel_s[:, 2 * c:2 * c + 2], hsel,
                        ksc_s[:, c:c + 1])
                # ---- WoKV[c,co] = VKT_c^T @ Wo_block (one-time) ----
                for c in range(NC_E):
                    for co in range(NC_E):
                        wps = accp.tile([128, 128], dt.float32, tag="wokv",
                                        bufs=1)
                        nc.tensor.matmul(
                            wps[:], vkt_s[:, c * 128:(c + 1) * 128],
                            wo_s[:, c * E + co * 128: c * E + (co + 1) * 128],
                            start=True, stop=True)
                        nc.vector.tensor_copy(
                            out=wokv_s[:, (c * NC_E + co) * 128:
                                       (c * NC_E + co + 1) * 128],
                            in_=wps[:])

            # =========================== PHASE B ==========================
            with tc.tile_pool(name="pb_ps", bufs=3, space="PSUM") as pbps, \
                 tc.tile_pool(name="pb_bc", bufs=2, space="PSUM") as pbbc, \
                 tc.tile_pool(name="pb_st", bufs=2, space="PSUM") as pbst, \
                 tc.tile_pool(name="pb_zd", bufs=1, space="PSUM") as pbzd, \
                 tc.tile_pool(name="pb_sb", bufs=3, space="SBUF") as pbsb, \
                 tc.tile_pool(name="pb_q", bufs=6, space="SBUF") as pbq, \
                 tc.tile_pool(name="pb_x1", bufs=6, space="SBUF") as pbx1, \
                 tc.tile_pool(name="pb_h", bufs=16, space="SBUF") as pbh, \
                 tc.tile_pool(name="pb_o", bufs=6, space="SBUF") as pbo:

                state = {}

                def ln_smalls(stat):
                    inv = 1.0 / E
                    mean = pbsb.tile([1, TT], dt.float32, tag="mean", bufs=2)
                    nc.vector.tensor_scalar_mul(mean[:], stat[0:1, :], inv)
                    msq = pbsb.tile([1, TT], dt.float32, tag="msq", bufs=2)
                    nc.vector.tensor_mul(msq[:], mean[:], mean[:])
                    var = pbsb.tile([1, TT], dt.float32, tag="var", bufs=2)
                    nc.vector.scalar_tensor_tensor(
                        out=var[:], in0=stat[32:33, :], scalar=inv,
                        in1=msq[:], op0=ALU.mult, op1=ALU.subtract)
                    rsf = pbsb.tile([1, TT], dt.float32, tag="rsf", bufs=2)
                    nc.vector.reciprocal_approx_fast(out=rsf[:], in_=var[:])
                    rs_b = pbsb.tile([1, TT], dt.bfloat16, tag="rsb", bufs=2)
                    nc.scalar.activation(rs_b[:], rsf[:], AF.Sqrt)
                    mean_b = pbsb.tile([1, TT], dt.bfloat16, tag="meanb",
                                       bufs=2)
                    nc.scalar.activation(mean_b[:], mean[:], AF.Copy)
                    return mean_b, rs_b

                def ln_finish(mean_b, rs_b, hts, g_c, be_c, opool, otag):
                    mbp = pbbc.tile([128, TT], dt.float32, tag="bc")
                    nc.tensor.matmul(mbp[:], ones1x128, mean_b[:],
                                     start=True, stop=True)
                    rbp = pbbc.tile([128, TT], dt.float32, tag="bc")
                    nc.tensor.matmul(rbp[:], ones1x128, rs_b[:],
                                     start=True, stop=True)
                    # bf16 SBUF copies so the per-chunk sub/mul get 2x DVE
                    mb = pbsb.tile([128, TT], dt.bfloat16, tag="mbb", bufs=2)
                    nc.scalar.activation(mb[:], mbp[:], AF.Copy)
                    rb = pbsb.tile([128, TT], dt.bfloat16, tag="rbb", bufs=2)
                    nc.scalar.activation(rb[:], rbp[:], AF.Copy)
                    outs = []
                    for c in range(len(hts)):
                        tmp = pbsb.tile([128, TT], dt.bfloat16, tag="nrm")
                        nc.vector.tensor_sub(tmp[:], hts[c][:], mb[:])
                        nc.vector.tensor_mul(tmp[:], tmp[:], rb[:])
                        o = opool.tile([128, TT], dt.bfloat16, tag=otag)
                        nc.scalar.activation(o[:], tmp[:], AF.Identity,
                                             bias=be_c(c), scale=g_c(c))
                        outs.append(o)
                    return outs

                def s1q_qproj(t):
                    t0 = t * TT
                    # -- Q = elu(Wq^T xT + bq)+1, just-in-time --
                    qts = []
                    for co in range(NC_E):
                        qps = pbps.tile([128, TT], dt.float32, tag="mm")
                        for ci in range(NC_E):
                            nc.tensor.matmul(
                                qps[:],
                                wq_s[:, ci * E + co * 128:
                                     ci * E + (co + 1) * 128],
                                xt_s[ci][:, t0:t0 + TT],
                                start=(ci == 0), stop=(ci == NC_E - 1))
                        t1 = pbsb.tile([128, TT], dt.bfloat16, tag="t1")
                        qt = pbq.tile([128, TT], dt.bfloat16, tag="qt",
                                      bufs=12)
                        nc.scalar.activation(t1[:], qps[:], AF.Relu,
                                             bias=bq_c(co))
                        nc.vector.tensor_scalar(
                            out=qt[:], in0=qps[:], scalar1=bq_c(co),
                            scalar2=0.0, op0=ALU.add, op1=ALU.min)
                        nc.scalar.activation(qt[:], qt[:], AF.Exp)
                        nc.vector.tensor_add(qt[:], qt[:], t1[:])
                        qts.append(qt)
                    state.setdefault(t, {})["qts"] = qts

                def s1z_attn_wo(t):
                    t0 = t * TT
                    qts = state[t]["qts"]
                    # -- Z denominators, packed; one fast reciprocal --
                    zden = pbzd.tile([128, TT], dt.float32, tag="zd")
                    for c in range(NC_E):
                        nc.tensor.matmul(zden[32 * c:32 * c + 2, :],
                                         ksel_s[:, 2 * c:2 * c + 2],
                                         qts[c][:],
                                         start=True, stop=True,
                                         skip_group_check=True,
                                         tile_position=(0, 32 * c))
                    zrf = pbsb.tile([128, TT], dt.float32, tag="zrf", bufs=2)
                    nc.vector.reciprocal_approx_fast(out=zrf[:], in_=zden[:])
                    zrb = pbsb.tile([128, TT], dt.bfloat16, tag="zrb", bufs=2)
                    nc.scalar.activation(zrb[:], zrf[:], AF.Copy)
                    # -- qz = Q * Z (broadcast via hexp4 matmul) --
                    qzs = []
                    for c in range(NC_E):
                        zb = pbbc.tile([128, TT], dt.float32, tag="bc")
                        nc.tensor.matmul(zb[:],
                                         hexp4_s[32 * c:32 * c + 2, :],
                                         zrb[32 * c:32 * c + 2, :],
                                         start=True, stop=True,
                                         tile_position=(32 * c, 0))
                        qz = pbq.tile([128, TT], dt.bfloat16, tag="qz")
                        nc.vector.tensor_mul(qz[:], qts[c][:], zb[:])
                        qzs.append(qz)
                    # -- attn @ Wo fused via WoKV + residual + LN1 stats --
                    h1ts = []
                    stat1 = pbst.tile([128, TT], dt.float32, tag="st")
                    for co in range(NC_E):
                        ops_ = pbps.tile([128, TT], dt.float32, tag="mm")
                        for c in range(NC_E):
                            nc.tensor.matmul(
                                ops_[:],
                                wokv_s[:, (c * NC_E + co) * 128:
                                       (c * NC_E + co + 1) * 128],
                                qzs[c][:],
                                start=(c == 0), stop=(c == NC_E - 1))
                        h1t = pbx1.tile([128, TT], dt.bfloat16, tag="h1",
                                        bufs=4)
                        nc.vector.scalar_tensor_tensor(
                            out=h1t[:], in0=ops_[:], scalar=bo_c(co),
                            in1=xt_s[co][:, t0:t0 + TT],
                            op0=ALU.add, op1=ALU.add)
                        h1ts.append(h1t)
                        sq = pbsb.tile([128, TT], dt.bfloat16, tag="sq",
                                       bufs=4)
                        nc.scalar.activation(sq[:], h1t[:], AF.Square)
                        nc.tensor.matmul(stat1[0:1, :], onesc, h1t[:],
                                         start=(co == 0),
                                         stop=(co == NC_E - 1),
                                         skip_group_check=True,
                                         tile_position=(0, 0))
                        nc.tensor.matmul(stat1[32:33, :], onesc, sq[:],
                                         start=(co == 0),
                                         stop=(co == NC_E - 1),
                                         skip_group_check=True,
                                         tile_position=(0, 32))
                    mb1, rb1 = ln_smalls(stat1)
                    state[t].update(h1ts=h1ts, mb1=mb1, rb1=rb1)

                def s2_ln1(t):
                    st = state[t]
                    st["x1ts"] = ln_finish(st["mb1"], st["rb1"], st["h1ts"],
                                           g1_c, be1_c, pbx1, "x1")

                def s3_ffn1(t):
                    x1ts = state[t]["x1ts"]
                    hts = []
                    for j in range(NC_H):
                        hps = pbps.tile([128, TT], dt.float32, tag="mm")
                        for ci in range(NC_E):
                            nc.tensor.matmul(
                                hps[:],
                                w1_s[:, ci * HID + j * 128:
                                     ci * HID + (j + 1) * 128],
                                x1ts[ci][:],
                                start=(ci == 0), stop=(ci == NC_E - 1))
                        ht = pbh.tile([128, TT], dt.bfloat16, tag="ht")
                        nc.scalar.activation(ht[:], hps[:], AF.Relu,
                                             bias=b1_c(j))
                        hts.append(ht)
                    state[t]["hts"] = hts

                def s4_ffn2(t, co_lo, co_hi):
                    x1ts = state[t]["x1ts"]
                    hts = state[t]["hts"]
                    if co_lo == 0:
                        state[t]["h2ts"] = []
                        state[t]["stat2"] = pbst.tile([128, TT], dt.float32,
                                                      tag="st", name="stat2")
                    h2ts = state[t]["h2ts"]
                    stat2 = state[t]["stat2"]
                    for co in range(co_lo, co_hi):
                        ops2 = pbps.tile([128, TT], dt.float32, tag="mm")
                        for j in range(NC_H):
                            nc.tensor.matmul(
                                ops2[:],
                                w2_s[:, j * E + co * 128:
                                     j * E + (co + 1) * 128],
                                hts[j][:],
                                start=(j == 0), stop=(j == NC_H - 1))
                        h2t = pbo.tile([128, TT], dt.bfloat16, tag="h2")
                        nc.vector.scalar_tensor_tensor(
                            out=h2t[:], in0=ops2[:], scalar=b2_c(co),
                            in1=x1ts[co][:], op0=ALU.add, op1=ALU.add)
                        h2ts.append(h2t)
                        sq = pbsb.tile([128, TT], dt.bfloat16, tag="sq",
                                       bufs=4)
                        nc.scalar.activation(sq[:], h2t[:], AF.Square)
                        nc.tensor.matmul(stat2[0:1, :], onesc, h2t[:],
                                         start=(co == 0),
                                         stop=(co == NC_E - 1),
                                         skip_group_check=True,
                                         tile_position=(0, 0))
                        nc.tensor.matmul(stat2[32:33, :], onesc, sq[:],
                                         start=(co == 0),
                                         stop=(co == NC_E - 1),
                                         skip_group_check=True,
                                         tile_position=(0, 32))
                    if co_hi == NC_E:
                        mb2, rb2 = ln_smalls(stat2)
                        state[t].update(mb2=mb2, rb2=rb2)

                def s5_out(t):
                    t0 = t * TT
                    st = state[t]
                    outs = ln_finish(st["mb2"], st["rb2"], st["h2ts"],
                                     g2_c, be2_c, pbo, "ou")
                    for c in range(NC_E):
                        nc.sync.dma_start(
                            out=out_d[c * 128:(c + 1) * 128, t0:t0 + TT],
                            in_=outs[c][:])
                    del state[t]

                # software pipeline, 3 tiles deep: FFN(t) overlaps
                # attention/LN1(t+1); Q-proj(t+2) fills the PE while the
                # x1(t+1) normalize chain completes.
                s1q_qproj(0)
                s1q_qproj(1)
                s1z_attn_wo(0)
                s2_ln1(0)
                for t in range(NT):
                    s3_ffn1(t)
                    if t + 1 < NT:
                        s1z_attn_wo(t + 1)
                    s4_ffn2(t, 0, 2)
                    if t + 1 < NT:
                        s2_ln1(t + 1)
                    s4_ffn2(t, 2, NC_E)
                    if t + 2 < NT:
                        s1q_qproj(t + 2)
                    s5_out(t)

    nc.compile()
    return nc


def _aux_arrays():
    aux = np.zeros((128, 3), dtype=BF16)
    aux[0:64, 0] = BF16(1.0)
    aux[64:128, 1] = BF16(1.0)
    aux[:, 2] = BF16(1.0)
    hexp4 = np.zeros((128, 128), dtype=BF16)
    for c in range(4):
        hexp4[32 * c, 0:64] = BF16(1.0)
        hexp4[32 * c + 1, 64:128] = BF16(1.0)
    onesr = np.ones((1, TT), dtype=BF16)
    return aux, hexp4, onesr


def _chunk_weight(w, nchunks):
    """[nchunks*128, X] f32 -> [128, nchunks*X] bf16 in c-major free layout."""
    X = w.shape[1]
    return np.ascontiguousarray(
        w.reshape(nchunks, 128, X).transpose(1, 0, 2).reshape(128, nchunks * X)
    ).astype(BF16)


def kernel(**inputs):
    if "nc" not in _CACHE:
        _CACHE["nc"] = _build()
    nc = _CACHE["nc"]

    x = np.asarray(inputs["x"], dtype=F32)
    aux, hexp4, onesr = _aux_arrays()
    pp = np.zeros((128, 44), dtype=F32)
    for c in range(4):
        pp[:, c] = inputs["bq"][c * 128:(c + 1) * 128]
        pp[:, 4 + c] = inputs["bo"][c * 128:(c + 1) * 128]
        pp[:, 24 + c] = inputs["b2"][c * 128:(c + 1) * 128]
        pp[:, 28 + c] = inputs["g1"][c * 128:(c + 1) * 128]
        pp[:, 32 + c] = inputs["be1"][c * 128:(c + 1) * 128]
        pp[:, 36 + c] = inputs["g2"][c * 128:(c + 1) * 128]
        pp[:, 40 + c] = inputs["be2"][c * 128:(c + 1) * 128]
    for j in range(16):
        pp[:, 8 + j] = inputs["b1"][j * 128:(j + 1) * 128]
    bkv = np.stack([np.asarray(inputs["bk"], F32),
                    np.asarray(inputs["bv"], F32)]).astype(BF16)

    shared = {
        "wq": _chunk_weight(np.asarray(inputs["Wq"], F32), NC_E),
        "wk": _chunk_weight(np.asarray(inputs["Wk"], F32), NC_E),
        "wv": _chunk_weight(np.asarray(inputs["Wv"], F32), NC_E),
        "wo": _chunk_weight(np.asarray(inputs["Wo"], F32), NC_E),
        "w1": _chunk_weight(np.asarray(inputs["W1"], F32), NC_E),
        "w2": _chunk_weight(np.asarray(inputs["W2"], F32), NC_H),
        "pp": pp, "aux": aux, "hexp4": hexp4, "onesr": onesr,
        "bkv": bkv,
    }
    in_maps = []
    for b in range(NCORES):
        m = dict(shared)
        m["xt"] = np.ascontiguousarray(x[b].T).astype(BF16)
        in_maps.append(m)

    res = run_bass_kernel_spmd(nc, in_maps, core_ids=list(range(NCORES)),
                               **_CACHE.get("run_kwargs", {}))
    _CACHE["last"] = res
    outs = [np.asarray(res.results[b]["out"]).astype(F32).T
            for b in range(NCORES)]
    return np.stack(outs, axis=0)
